# revision 12
# baseline (speedup 1.0000x reference)
"""Trainium2 Bass kernel for nn_DGM_15839839388164 (retrieval_knn).

Sharding: 512 rows per core x 8 cores. Host pre-transposes x; each core gets
the full x.T (replicated) plus its own 512-column slice, packed with the
weights and small constants into ONE bundle tensor so that every matmul
operand has a single DMA producer (TRN2 LDWEIGHTS accepts only one sync wait).

Per core:
  x_      = x @ W_enc                      (row block, output)
  x_aux   = x @ W_emb                      (row block, output)
  q_ij    = sqrt(sq_i + sq_j - 2*(x_aux @ x_aux.T)_ij)  = -probs_ij
  row stats of probs -> pn = gamma*(probs-mean)/(std_ddof1+eps)
  top-16 of pn per row -> exact sort-based 1.5-entmax threshold tau*
    (support size is <= 9 for this input regime; prefix property of the
     indicator makes the truncation exact)
  qthr_i  = mean_q_i - (rowmax_i + 2*tau*_i)*(std_i+eps)/gamma
  AllGather(qthr);  adj_ij = q_ij < max(qthr_i, qthr_j)   (q is symmetric)
  logprobs = rowsum(adj)

The d2 diagonal is *exactly* 0.0 by construction: sq is extracted from the
PE's own diagonal block (S = -0.5*Block_ii, so psum_ii accumulates to -S and
the sqrt's +S bias cancels bit-exactly) -- no relu pass, no NaN. The diagonal
of pn is then mean_q*a (the unique row max), so sorted column 0 is a sentinel
that is dropped; host zeroes the adjacency diagonal and corrects logprobs with
the device-computed diagonal bit.
"""

import numpy as np

N, D, H = 4096, 256, 256
NCORES = 8
RB = N // NCORES          # 512 rows per core
NG = RB // 128            # 4 row-groups of 128
NCH = N // 512            # 8 free-dim chunks of 512
GAMMA = 10.0
EPS = 1e-6
K = 16                    # top-K extracted (2 rounds of max8); col0 = diag sentinel
KD = K - 1                # 15 usable sorted off-diag values

# bundle column layout
C_XT = 0                  # [0, 4096)       x.T full
C_XTM = C_XT + N          # [4096, 4608)    x.T my columns
C_WENC = C_XTM + RB       # [4608, 4864)    W_enc
C_WEMB = C_WENC + H       # [4864, 5120)    W_emb
C_EYE = C_WEMB + H        # [5120, 5248)    -0.5*I (rows 0..127 of half 0)
C_RHO = C_EYE + 128       # [5248, 5263)    1..15 (all rows)
C_ONES = C_RHO + KD       # [5263, 5391)    ones (row 0)
C_TOT = C_ONES + 128      # 5391

_BUILT = {}


def _build_nc():
    import concourse.bass as bass
    import concourse.mybir as mybir
    from concourse import bacc
    from concourse.tile import TileContext

    f32 = mybir.dt.float32
    i32 = mybir.dt.int32
    Alu = mybir.AluOpType
    Act = mybir.ActivationFunctionType

    nc = bacc.Bacc("TRN2", num_devices=NCORES)

    bun = nc.dram_tensor("bun", [D, C_TOT], f32, kind="ExternalInput")

    x_out = nc.dram_tensor("x_out", [RB, H], f32, kind="ExternalOutput")
    xa_out = nc.dram_tensor("xa_out", [RB, H], f32, kind="ExternalOutput")
    adj_out = nc.dram_tensor("adj_out", [RB, N], i32, kind="ExternalOutput")
    lp_out = nc.dram_tensor("lp_out", [RB, 1], i32, kind="ExternalOutput")

    with TileContext(nc) as tc:
        with (
            tc.tile_pool(name="const", bufs=1) as const,
            tc.tile_pool(name="persist", bufs=1) as persist,
            tc.tile_pool(name="stats", bufs=2) as stats,
            tc.tile_pool(name="small", bufs=4) as small,
            tc.tile_pool(name="psA", bufs=2, space="PSUM") as psA,
            tc.tile_pool(name="psG", bufs=4, space="PSUM") as psG,
            tc.tile_pool(name="dram", bufs=1, space="DRAM") as dram,
        ):
            sq_row = persist.tile([1, N], f32, name="sq_row")
            rho_sb = const.tile([128, KD], f32, name="rho_sb")
            irho_sb = const.tile([128, KD], f32, name="irho_sb")

            cc1_in = dram.tile([1, RB], f32, name="cc1_in")
            cc1_out = dram.tile([1, N], f32, name="cc1_out", addr_space="Shared")
            cc2_in = dram.tile([1, RB], f32, name="cc2_in")
            cc2_out = dram.tile([1, N], f32, name="cc2_out", addr_space="Shared")

            S_all = stats.tile([128, NG], f32, name="S_all", bufs=1)
            qthr_all = stats.tile([128, NG], f32, name="qthr_all", bufs=1)
            q_sb = [persist.tile([128, N], f32, name=f"q_sb{g}") for g in range(NG)]
            bnst = [stats.tile([128, NCH, 6], f32, name=f"bnst{g}", bufs=1)
                    for g in range(NG)]

            # ============ phase 1+2: matmuls, G row-block -> q, stats ============
            with tc.tile_pool(name="mats", bufs=1) as mats:
                xaTm = [mats.tile([128, RB], f32, name=f"xaTm{k}") for k in range(2)]
                xaTms = [mats.tile([128, RB], f32, name=f"xaTms{k}") for k in range(2)]
                xaTs = [mats.tile([128, N], f32, name=f"xaTs{k}") for k in range(2)]
                bsb = [mats.tile([128, C_TOT], f32, name=f"bsb{k}") for k in range(2)]
                for k in range(2):
                    nc.gpsimd.dma_start(out=bsb[k], in_=bun[128 * k:128 * (k + 1), :])
                xT_sb = [b[:, C_XT:C_XT + N] for b in bsb]
                xTm_sb = [b[:, C_XTM:C_XTM + RB] for b in bsb]
                wenc_sb = [b[:, C_WENC:C_WENC + H] for b in bsb]
                wemb_sb = [b[:, C_WEMB:C_WEMB + H] for b in bsb]
                eyeh_sb = bsb[0][:, C_EYE:C_EYE + 128]
                ones_row = bsb[0][0:1, C_ONES:C_ONES + 128]
                nc.vector.tensor_copy(out=rho_sb, in_=bsb[0][:, C_RHO:C_RHO + KD])
                nc.vector.reciprocal(out=irho_sb, in_=rho_sb)

                # x_ / x_aux row blocks
                for g in range(NG):
                    xps = psA.tile([128, H], f32, name="xps", tag="psa")
                    for k in range(2):
                        nc.tensor.matmul(
                            xps, xTm_sb[k][:, 128 * g:128 * (g + 1)], wenc_sb[k],
                            start=(k == 0), stop=(k == 1),
                        )
                    xrow = small.tile([128, H], f32, name="xrow")
                    nc.scalar.copy(out=xrow, in_=xps)
                    nc.sync.dma_start(out=x_out[128 * g:128 * (g + 1), :], in_=xrow)
                    aps = psA.tile([128, H], f32, name="aps", tag="psa")
                    for k in range(2):
                        nc.tensor.matmul(
                            aps, xTm_sb[k][:, 128 * g:128 * (g + 1)], wemb_sb[k],
                            start=(k == 0), stop=(k == 1),
                        )
                    arow = small.tile([128, H], f32, name="arow")
                    nc.scalar.copy(out=arow, in_=aps)
                    nc.sync.dma_start(out=xa_out[128 * g:128 * (g + 1), :], in_=arow)

                # xa.T for my columns: unscaled (lhsT for G) and -2 scaled
                for hh in range(2):
                    mps = psA.tile([128, RB], f32, name="mps", tag="psa")
                    for k in range(2):
                        nc.tensor.matmul(
                            mps, wemb_sb[k][:, 128 * hh:128 * (hh + 1)], xTm_sb[k],
                            start=(k == 0), stop=(k == 1),
                        )
                    nc.scalar.copy(out=xaTm[hh], in_=mps)
                    nc.scalar.mul(out=xaTms[hh], in_=mps, mul=-2.0)

                # S = -0.5 * diag(xaTm.T @ xaTms)  (= sq in PE arithmetic)
                for g in range(NG):
                    dps = psA.tile([128, 128], f32, name="dps", tag="psa")
                    for k in range(2):
                        nc.tensor.matmul(
                            dps, xaTm[k][:, 128 * g:128 * (g + 1)],
                            xaTms[k][:, 128 * g:128 * (g + 1)],
                            start=(k == 0), stop=(k == 1),
                        )
                    dd = small.tile([128, 128], f32, name="dd")
                    nc.scalar.copy(out=dd, in_=dps)
                    ddm = small.tile([128, 128], f32, name="ddm")
                    nc.vector.tensor_mul(ddm, dd, eyeh_sb)
                    junk = small.tile([128, 128], f32, name="junk")
                    nc.scalar.activation(
                        out=junk, in_=ddm, func=Act.Identity,
                        accum_out=S_all[:, g:g + 1],
                    )
                nc.gpsimd.dma_start(
                    out=bass.AP(tensor=cc1_in[:].tensor, offset=cc1_in[:].offset,
                                ap=[[1, 128], [128, NG]]),
                    in_=S_all,
                )
                nc.gpsimd.collective_compute(
                    "AllGather", Alu.bypass,
                    replica_groups=[list(range(NCORES))],
                    ins=[cc1_in[:].opt()], outs=[cc1_out[:].opt()],
                )
                nc.gpsimd.dma_start(out=sq_row, in_=cc1_out[:])

                # full xa.T, -2 scaled (moving operand for G)
                for hh in range(2):
                    for j in range(NCH):
                        nps = psG.tile([128, 512], f32, name="nps", tag="psg")
                        for k in range(2):
                            nc.tensor.matmul(
                                nps, wemb_sb[k][:, 128 * hh:128 * (hh + 1)],
                                xT_sb[k][:, 512 * j:512 * (j + 1)],
                                start=(k == 0), stop=(k == 1),
                            )
                        nc.scalar.mul(
                            out=xaTs[hh][:, 512 * j:512 * (j + 1)], in_=nps, mul=-2.0,
                        )

                # G row-block -> q tiles + bn stats
                for g in range(NG):
                    for j in range(NCH):
                        gps = psG.tile([128, 512], f32, name="gps", tag="psg")
                        for k in range(2):
                            nc.tensor.matmul(
                                gps, xaTm[k][:, 128 * g:128 * (g + 1)],
                                xaTs[k][:, 512 * j:512 * (j + 1)],
                                start=(k == 0), stop=False,
                            )
                        nc.tensor.matmul(
                            gps, ones_row, sq_row[0:1, 512 * j:512 * (j + 1)],
                            start=False, stop=True,
                        )
                        qch = q_sb[g][:, 512 * j:512 * (j + 1)]
                        nc.scalar.activation(
                            out=qch, in_=gps, func=Act.Sqrt,
                            bias=S_all[:, g:g + 1], scale=1.0,
                        )
                        nc.vector.bn_stats(out=bnst[g][:, j, :], in_=qch)

            # ============ phase 2b: row stats + entmax threshold ============
            with tc.tile_pool(name="ph2", bufs=1) as ph2:
                for g in range(NG):
                    mv = stats.tile([128, 2], f32, name="mv")
                    nc.vector.bn_aggr(out=mv, in_=bnst[g])
                    mu = mv[:, 0:1]
                    sig = stats.tile([128, 1], f32, name="sig")  # std(ddof=1) + eps
                    nc.scalar.activation(
                        out=sig, in_=mv[:, 1:2], func=Act.Sqrt,
                        scale=float(N) / float(N - 1),
                    )
                    nc.vector.tensor_scalar(
                        out=sig, in0=sig, scalar1=EPS, scalar2=None, op0=Alu.add,
                    )
                    a_r = stats.tile([128, 1], f32, name="a_r")  # gamma/(std+eps)
                    nc.vector.reciprocal(out=a_r, in_=sig)
                    nc.vector.tensor_scalar(
                        out=a_r, in0=a_r, scalar1=GAMMA, scalar2=None, op0=Alu.mult,
                    )
                    nega = stats.tile([128, 1], f32, name="nega")
                    nc.vector.tensor_scalar(
                        out=nega, in0=a_r, scalar1=-1.0, scalar2=None, op0=Alu.mult,
                    )
                    mua = stats.tile([128, 1], f32, name="mua")
                    nc.vector.tensor_mul(mua, mu, a_r)

                    # pn = (-a)*q + mu*a  (diag q=0 -> pn=mu*a, the unique row max)
                    pn = ph2.tile([128, N], f32, name="pn")
                    nc.scalar.activation(
                        out=pn, in_=q_sb[g], func=Act.Identity, bias=mua, scale=nega,
                    )
                    s16 = small.tile([128, K], f32, name="s16")
                    nc.vector.max(out=s16[:, 0:8], in_=pn)
                    pn2 = ph2.tile([128, N], f32, name="pn2")
                    nc.vector.match_replace(
                        out=pn2, in_to_replace=s16[:, 0:8], in_values=pn,
                        imm_value=-1e30,
                    )
                    nc.vector.max(out=s16[:, 8:16], in_=pn2)

                    zs = s16[:, 1:K]          # sorted top-15 off-diag pn
                    rowmax = zs[:, 0:1]
                    xk = small.tile([128, KD], f32, name="xk")
                    nc.vector.tensor_scalar(
                        out=xk, in0=zs, scalar1=rowmax, scalar2=0.5,
                        op0=Alu.subtract, op1=Alu.mult,
                    )
                    xk2 = small.tile([128, KD], f32, name="xk2")
                    nc.vector.tensor_mul(xk2, xk, xk)
                    cs1 = small.tile([128, KD], f32, name="cs1")
                    nc.vector.tensor_tensor_scan(
                        out=cs1, data0=xk, data1=xk, initial=0.0,
                        op0=Alu.add, op1=Alu.bypass,
                    )
                    cs2 = small.tile([128, KD], f32, name="cs2")
                    nc.vector.tensor_tensor_scan(
                        out=cs2, data0=xk2, data1=xk2, initial=0.0,
                        op0=Alu.add, op1=Alu.bypass,
                    )
                    m_t = small.tile([128, KD], f32, name="m_t")
                    nc.vector.tensor_mul(m_t, cs1, irho_sb)
                    msq = small.tile([128, KD], f32, name="msq")
                    nc.vector.tensor_mul(msq, cs2, irho_sb)
                    mm_ = small.tile([128, KD], f32, name="mm_")
                    nc.vector.tensor_mul(mm_, m_t, m_t)
                    ss_ = small.tile([128, KD], f32, name="ss_")
                    nc.vector.tensor_sub(ss_, msq, mm_)
                    nc.vector.tensor_mul(ss_, ss_, rho_sb)
                    dl = small.tile([128, KD], f32, name="dl")
                    nc.scalar.activation(
                        out=dl, in_=ss_, func=Act.Identity, bias=1.0, scale=-1.0,
                    )
                    nc.vector.tensor_mul(dl, dl, irho_sb)
                    nc.vector.tensor_scalar(
                        out=dl, in0=dl, scalar1=0.0, scalar2=None, op0=Alu.max,
                    )
                    sq_d = small.tile([128, KD], f32, name="sq_d")
                    nc.scalar.activation(out=sq_d, in_=dl, func=Act.Sqrt)
                    tau = small.tile([128, KD], f32, name="tau")
                    nc.vector.tensor_sub(tau, m_t, sq_d)
                    ind = small.tile([128, KD], f32, name="ind")
                    nc.vector.tensor_tensor(out=ind, in0=tau, in1=xk, op=Alu.is_le)
                    dsel = small.tile([128, KD], f32, name="dsel")
                    nc.vector.tensor_sub(
                        dsel[:, 0:KD - 1], ind[:, 0:KD - 1], ind[:, 1:KD],
                    )
                    nc.vector.tensor_copy(out=dsel[:, KD - 1:KD], in_=ind[:, KD - 1:KD])
                    tsel = small.tile([128, KD], f32, name="tsel")
                    nc.vector.tensor_mul(tsel, tau, dsel)
                    tau_s = stats.tile([128, 1], f32, name="tau_s")
                    nc.vector.tensor_reduce(
                        out=tau_s, in_=tsel, axis=mybir.AxisListType.X, op=Alu.add,
                    )
                    u_t = stats.tile([128, 1], f32, name="u_t")
                    nc.vector.tensor_scalar(
                        out=u_t, in0=tau_s, scalar1=2.0, scalar2=rowmax,
                        op0=Alu.mult, op1=Alu.add,
                    )
                    nc.vector.tensor_mul(u_t, u_t, sig)
                    nc.vector.tensor_scalar(
                        out=qthr_all[:, g:g + 1], in0=u_t, scalar1=-1.0 / GAMMA,
                        scalar2=mu, op0=Alu.mult, op1=Alu.add,
                    )

            nc.gpsimd.dma_start(
                out=bass.AP(tensor=cc2_in[:].tensor, offset=cc2_in[:].offset,
                            ap=[[1, 128], [128, NG]]),
                in_=qthr_all,
            )
            nc.gpsimd.collective_compute(
                "AllGather", Alu.bypass,
                replica_groups=[list(range(NCORES))],
                ins=[cc2_in[:].opt()], outs=[cc2_out[:].opt()],
            )

            # ============ phase 3: adjacency + logprobs ============
            with tc.tile_pool(name="ph3", bufs=2) as ph3:
                qthr_b = ph3.tile([128, N], f32, name="qthr_b", bufs=1)
                nc.gpsimd.dma_start(
                    out=qthr_b,
                    in_=bass.AP(tensor=cc2_out[:].tensor, offset=cc2_out[:].offset,
                                ap=[[0, 128], [1, N]]),
                )
                for g in range(NG):
                    thr = ph3.tile([128, N], f32, name="thr", bufs=1)
                    nc.vector.tensor_scalar(
                        out=thr, in0=qthr_b, scalar1=qthr_all[:, g:g + 1],
                        scalar2=None, op0=Alu.max,
                    )
                    b_f = ph3.tile([128, N], f32, name="b_f")
                    nc.vector.tensor_tensor(out=b_f, in0=q_sb[g], in1=thr, op=Alu.is_lt)
                    adj_t = ph3.tile([128, N], i32, name="adj_t")
                    lp_f = stats.tile([128, 1], f32, name="lp_f")
                    nc.scalar.activation(
                        out=adj_t, in_=b_f, func=Act.Copy, accum_out=lp_f,
                    )
                    lp_i = stats.tile([128, 1], i32, name="lp_i")
                    nc.vector.tensor_copy(out=lp_i, in_=lp_f)
                    nc.sync.dma_start(out=adj_out[128 * g:128 * (g + 1), :], in_=adj_t)
                    nc.sync.dma_start(out=lp_out[128 * g:128 * (g + 1), :], in_=lp_i)

    nc.finalize()
    return nc


def _get_nc():
    if "nc" not in _BUILT:
        _BUILT["nc"] = _build_nc()
    return _BUILT["nc"]


def _make_bundle(xT, xTm, W_enc, W_emb):
    bun = np.zeros((D, C_TOT), dtype=np.float32)
    bun[:, C_XT:C_XT + N] = xT
    bun[:, C_XTM:C_XTM + RB] = xTm
    bun[:, C_WENC:C_WENC + H] = W_enc
    bun[:, C_WEMB:C_WEMB + H] = W_emb
    bun[0:128, C_EYE:C_EYE + 128] = np.eye(128, dtype=np.float32) * np.float32(-0.5)
    bun[:, C_RHO:C_RHO + KD] = np.arange(1, KD + 1, dtype=np.float32)[None, :]
    bun[0, C_ONES:C_ONES + 128] = 1.0
    return bun


def kernel(**inputs):
    x = np.ascontiguousarray(np.asarray(inputs["x"], dtype=np.float32))
    W_enc = np.ascontiguousarray(np.asarray(inputs["W_enc"], dtype=np.float32))
    W_emb = np.ascontiguousarray(np.asarray(inputs["W_emb"], dtype=np.float32))
    assert x.shape == (N, D)

    from concourse.bass_utils import run_bass_kernel_spmd

    xT = np.ascontiguousarray(x.T)
    in_maps = []
    for c in range(NCORES):
        in_maps.append({
            "bun": _make_bundle(xT, xT[:, c * RB:(c + 1) * RB], W_enc, W_emb),
        })

    nc = _get_nc()
    res = run_bass_kernel_spmd(nc, in_maps, core_ids=list(range(NCORES)))
    _BUILT["last_results"] = res
    outs = res.results

    x_ = np.concatenate([outs[c]["x_out"] for c in range(NCORES)], axis=0)
    x_aux = np.concatenate([outs[c]["xa_out"] for c in range(NCORES)], axis=0)
    adj = np.concatenate([outs[c]["adj_out"] for c in range(NCORES)], axis=0)
    lp = np.concatenate([outs[c]["lp_out"] for c in range(NCORES)], axis=0).reshape(N)

    diag = np.ascontiguousarray(np.diagonal(adj)).astype(np.int32)
    logprobs = (lp - diag).astype(np.int32)
    np.fill_diagonal(adj, 0)
    return x_, x_aux, adj, logprobs


# revision 16
# speedup vs baseline: 1.1555x; 1.1555x over previous
"""Trainium2 Bass kernel for nn_DGM_15839839388164 (retrieval_knn).

Sharding: 512 rows per core x 8 cores. Host pre-transposes x; each core gets
the full x.T (replicated) plus its own 512-column slice, packed with the
weights and small constants into ONE bundle tensor so every matmul operand
has a single DMA producer.

Per core:
  x_      = x @ W_enc                      (row block, output)
  x_aux   = x @ W_emb                      (row block, output)
  q_ij    = sqrt(sq_i + sq_j - 2*(x_aux @ x_aux.T)_ij)  = -probs_ij
  row stats of probs -> pn = gamma*(probs-mean)/(std_ddof1+eps)
  top-16 of pn per row -> exact sort-based 1.5-entmax threshold tau*
    (support <= 9 for this input regime; the indicator's prefix property
     makes the top-15 truncation exact)
  qthr_i  = mean_q_i - (rowmax_i + 2*tau*_i)*(std_i+eps)/gamma
  AllGather(qthr);  adj_ij = q_ij < max(qthr_i, qthr_j)   (q is symmetric)
  logprobs = rowsum(adj)

Performance structure:
  - sq_j enters the distance matrix through two K=1 fp16 "extras" matmuls
    (sq split exactly into fp16 hi+lo; max residual 3e-5, verified zero
    adjacency flips) -- ~4x cheaper than an fp32 extras pass.
  - The sqrt bias mirrors the PSUM accumulation bit-exactly on DVE, so the
    d2 diagonal is *exactly* 0.0 -- no relu pass, no NaN. pn's diagonal is
    then mean_q*a (the unique row max); sorted col 0 is a dropped sentinel.
  - The entmax threshold chain runs batched over all 4 row-groups
    ([128, 4, 15] tiles) to avoid serial tiny-op latency.
  - Host zeroes the adjacency diagonal and corrects logprobs with the
    device-computed diagonal bit.
"""

import numpy as np

N, D, H = 4096, 256, 256
NCORES = 8
RB = N // NCORES          # 512 rows per core
NG = RB // 128            # 4 row-groups of 128
NCH = N // 512            # 8 free-dim chunks of 512
GAMMA = 10.0
EPS = 1e-6
K = 16                    # top-K extracted (2 rounds of max8); col0 = diag sentinel
KD = K - 1                # 15 usable sorted off-diag values

# bundle column layout
C_XT = 0                  # x.T full
C_XTM = C_XT + N          # x.T my columns
C_WENC = C_XTM + RB
C_WEMB = C_WENC + H
C_EYE = C_WEMB + H        # -0.5*I (rows 0..127 of half 0)
C_RHO = C_EYE + 128       # tile(1..15, NG)  (all rows)
C_TOT = C_RHO + NG * KD

_BUILT = {}


def _build_nc():
    import concourse.bass as bass
    import concourse.mybir as mybir
    from concourse import bacc
    from concourse.tile import TileContext

    f32 = mybir.dt.float32
    f16 = mybir.dt.float16
    i32 = mybir.dt.int32
    Alu = mybir.AluOpType
    Act = mybir.ActivationFunctionType

    nc = bacc.Bacc("TRN2", num_devices=NCORES, dynamic_dma_scratch_size=8192)

    bun = nc.dram_tensor("bun", [D, C_TOT], f32, kind="ExternalInput")

    x_out = nc.dram_tensor("x_out", [RB, H], f32, kind="ExternalOutput")
    xa_out = nc.dram_tensor("xa_out", [RB, H], f32, kind="ExternalOutput")
    adj_out = nc.dram_tensor("adj_out", [RB, N], i32, kind="ExternalOutput")
    lp_out = nc.dram_tensor("lp_out", [RB, 1], i32, kind="ExternalOutput")

    with TileContext(nc) as tc:
        with (
            tc.tile_pool(name="const", bufs=1) as const,
            tc.tile_pool(name="persist", bufs=1) as persist,
            tc.tile_pool(name="stats", bufs=2) as stats,
            tc.tile_pool(name="small", bufs=4) as small,
            tc.tile_pool(name="psA", bufs=2, space="PSUM") as psA,
            tc.tile_pool(name="psG", bufs=6, space="PSUM") as psG,
            tc.tile_pool(name="dram", bufs=1, space="DRAM") as dram,
        ):
            rho_sb = const.tile([128, NG, KD], f32, name="rho_sb")
            irho_sb = const.tile([128, NG, KD], f32, name="irho_sb")
            ones16 = const.tile([1, 128], f16, name="ones16")
            nc.vector.memset(ones16, 1.0)

            cc1_in = dram.tile([1, 2 * RB], f16, name="cc1_in")
            cc1_out = dram.tile([1, 2 * N], f16, name="cc1_out", addr_space="Shared")
            cc2_in = dram.tile([1, RB], f32, name="cc2_in")
            cc2_out = dram.tile([1, N], f32, name="cc2_out", addr_space="Shared")

            S_all = stats.tile([128, NG], f32, name="S_all", bufs=1)
            negu = stats.tile([128, NG], f32, name="negu", bufs=1)  # sqrt bias
            qthr_all = stats.tile([128, NG], f32, name="qthr_all", bufs=1)
            s16_all = stats.tile([128, NG, K], f32, name="s16_all", bufs=1)
            mvall = stats.tile([128, NG, 2], f32, name="mvall", bufs=1)
            sig_all = stats.tile([128, NG], f32, name="sig_all", bufs=1)
            q_sb = [persist.tile([128, N], f32, name=f"q_sb{g}") for g in range(NG)]
            bnst = [stats.tile([128, NCH, 6], f32, name=f"bnst{g}", bufs=1)
                    for g in range(NG)]

            # ============ phase 1: matmuls, S/AllGather, G -> q ============
            with tc.tile_pool(name="mats", bufs=1) as mats:
                xaTm = [mats.tile([128, RB], f32, name=f"xaTm{k}") for k in range(2)]
                xaTms = [mats.tile([128, RB], f32, name=f"xaTms{k}") for k in range(2)]
                xaTs = [mats.tile([128, N], f32, name=f"xaTs{k}") for k in range(2)]
                sq16 = mats.tile([1, 2 * N], f16, name="sq16")
                bpool_cm = tc.tile_pool(name="bpool", bufs=1)
                bpool = bpool_cm.__enter__()
                bsb = [bpool.tile([128, C_TOT], f32, name=f"bsb{k}") for k in range(2)]
                for k in range(2):
                    nc.gpsimd.dma_start(out=bsb[k], in_=bun[128 * k:128 * (k + 1), :])
                xT_sb = [b[:, C_XT:C_XT + N] for b in bsb]
                xTm_sb = [b[:, C_XTM:C_XTM + RB] for b in bsb]
                wenc_sb = [b[:, C_WENC:C_WENC + H] for b in bsb]
                wemb_sb = [b[:, C_WEMB:C_WEMB + H] for b in bsb]
                eyeh_sb = bsb[0][:, C_EYE:C_EYE + 128]
                rho_src = bsb[0][:, C_RHO:C_RHO + NG * KD]
                nc.vector.tensor_copy(
                    out=rho_sb, in_=rho_src.rearrange("p (g k) -> p g k", g=NG),
                )
                nc.vector.reciprocal(out=irho_sb, in_=rho_sb)

                # xa.T for my columns: unscaled (lhsT for G) and -2 scaled
                for hh in range(2):
                    mps = psA.tile([128, RB], f32, name="mps", tag="psa")
                    for k in range(2):
                        nc.tensor.matmul(
                            mps, wemb_sb[k][:, 128 * hh:128 * (hh + 1)], xTm_sb[k],
                            start=(k == 0), stop=(k == 1),
                        )
                    nc.scalar.copy(out=xaTm[hh], in_=mps)
                    nc.scalar.mul(out=xaTms[hh], in_=mps, mul=-2.0)

                # S = -0.5 * diag(xaTm.T @ xaTms) = sq in PE arithmetic;
                # split S into exact fp16 hi+lo; mirror the PSUM accumulation
                # (-2S + hi + lo) on DVE so the sqrt bias zeroes the diagonal.
                h16 = stats.tile([128, NG], f16, name="h16", bufs=1)
                l16 = stats.tile([128, NG], f16, name="l16", bufs=1)
                for g in range(NG):
                    dps = psA.tile([128, 128], f32, name="dps", tag="psa")
                    for k in range(2):
                        nc.tensor.matmul(
                            dps, xaTm[k][:, 128 * g:128 * (g + 1)],
                            xaTms[k][:, 128 * g:128 * (g + 1)],
                            start=(k == 0), stop=(k == 1),
                        )
                    dd = small.tile([128, 128], f32, name="dd", bufs=2)
                    nc.scalar.copy(out=dd, in_=dps)
                    ddm = small.tile([128, 128], f32, name="ddm", bufs=2)
                    nc.vector.tensor_mul(ddm, dd, eyeh_sb)
                    junk = small.tile([128, 128], f32, name="junk", bufs=1)
                    nc.scalar.activation(
                        out=junk, in_=ddm, func=Act.Identity,
                        accum_out=S_all[:, g:g + 1],
                    )
                    sg = S_all[:, g:g + 1]
                    nc.vector.tensor_copy(out=h16[:, g:g + 1], in_=sg)
                    h32 = small.tile([128, 1], f32, name="h32")
                    nc.vector.tensor_copy(out=h32, in_=h16[:, g:g + 1])
                    r1 = small.tile([128, 1], f32, name="r1")
                    nc.vector.tensor_sub(r1, sg, h32)
                    nc.vector.tensor_copy(out=l16[:, g:g + 1], in_=r1)
                    l32 = small.tile([128, 1], f32, name="l32")
                    nc.vector.tensor_copy(out=l32, in_=l16[:, g:g + 1])
                    # u = (-2S + h) + l ; bias = -u
                    u1 = small.tile([128, 1], f32, name="u1")
                    nc.vector.tensor_scalar(
                        out=u1, in0=sg, scalar1=-2.0, scalar2=None, op0=Alu.mult,
                    )
                    nc.vector.tensor_add(u1, u1, h32)
                    nc.vector.tensor_add(u1, u1, l32)
                    nc.vector.tensor_scalar(
                        out=negu[:, g:g + 1], in0=u1, scalar1=-1.0, scalar2=None,
                        op0=Alu.mult,
                    )
                nc.gpsimd.dma_start(
                    out=bass.AP(tensor=cc1_in[:].tensor, offset=cc1_in[:].offset,
                                ap=[[1, 128], [128, NG]]),
                    in_=h16,
                )
                nc.gpsimd.dma_start(
                    out=bass.AP(tensor=cc1_in[:].tensor,
                                offset=cc1_in[:].offset + RB,
                                ap=[[1, 128], [128, NG]]),
                    in_=l16,
                )
                nc.gpsimd.collective_compute(
                    "AllGather", Alu.bypass,
                    replica_groups=[list(range(NCORES))],
                    ins=[cc1_in[:].opt()], outs=[cc1_out[:].opt()],
                )
                nc.gpsimd.dma_start(out=sq16, in_=cc1_out[:])

                # full xa.T, -2 scaled (moving operand for G)
                for hh in range(2):
                    for j in range(NCH):
                        nps = psG.tile([128, 512], f32, name="nps", tag="psg")
                        for k in range(2):
                            nc.tensor.matmul(
                                nps, wemb_sb[k][:, 128 * hh:128 * (hh + 1)],
                                xT_sb[k][:, 512 * j:512 * (j + 1)],
                                start=(k == 0), stop=(k == 1),
                            )
                        nc.scalar.mul(
                            out=xaTs[hh][:, 512 * j:512 * (j + 1)], in_=nps, mul=-2.0,
                        )

                # x_ / x_aux row blocks
                for g in range(NG):
                    xps = psA.tile([128, H], f32, name="xps", tag="psa")
                    for k in range(2):
                        nc.tensor.matmul(
                            xps, xTm_sb[k][:, 128 * g:128 * (g + 1)], wenc_sb[k],
                            start=(k == 0), stop=(k == 1),
                        )
                    xrow = small.tile([128, H], f32, name="xrow", bufs=2)
                    nc.scalar.copy(out=xrow, in_=xps)
                    nc.sync.dma_start(out=x_out[128 * g:128 * (g + 1), :], in_=xrow)
                    aps = psA.tile([128, H], f32, name="aps", tag="psa")
                    for k in range(2):
                        nc.tensor.matmul(
                            aps, xTm_sb[k][:, 128 * g:128 * (g + 1)], wemb_sb[k],
                            start=(k == 0), stop=(k == 1),
                        )
                    arow = small.tile([128, H], f32, name="arow", bufs=2)
                    nc.scalar.copy(out=arow, in_=aps)
                    nc.sync.dma_start(out=xa_out[128 * g:128 * (g + 1), :], in_=arow)
                bpool_cm.__exit__(None, None, None)

                # G row-block -> q tiles + bn stats + per-rg pn/top16
                for g in range(NG):
                    for j in range(NCH):
                        gps = psG.tile([128, 512], f32, name="gps", tag="psg")
                        for k in range(2):
                            nc.tensor.matmul(
                                gps, xaTm[k][:, 128 * g:128 * (g + 1)],
                                xaTs[k][:, 512 * j:512 * (j + 1)],
                                start=(k == 0), stop=False,
                            )
                        nc.tensor.matmul(
                            gps, ones16, sq16[0:1, 1024 * j:1024 * j + 512],
                            start=False, stop=False, skip_group_check=True,
                        )
                        nc.tensor.matmul(
                            gps, ones16, sq16[0:1, 1024 * j + 512:1024 * j + 1024],
                            start=False, stop=True, skip_group_check=True,
                        )
                        qch = q_sb[g][:, 512 * j:512 * (j + 1)]
                        nc.scalar.activation(
                            out=qch, in_=gps, func=Act.Sqrt,
                            bias=negu[:, g:g + 1], scale=1.0,
                        )
                        nc.vector.bn_stats(out=bnst[g][:, j, :], in_=qch)

                    # per-row-group scalars + pn + top-16 (overlaps next rg's G)
                    nc.vector.bn_aggr(out=mvall[:, g, :], in_=bnst[g])
                    sgv = sig_all[:, g:g + 1]
                    nc.scalar.activation(
                        out=sgv, in_=mvall[:, g, 1:2], func=Act.Sqrt,
                        scale=float(N) / float(N - 1),
                    )
                    nc.vector.tensor_scalar(
                        out=sgv, in0=sgv, scalar1=EPS, scalar2=None, op0=Alu.add,
                    )
                    a_r = small.tile([128, 1], f32, name="a_r")
                    nc.vector.reciprocal(out=a_r, in_=sgv)
                    nc.vector.tensor_scalar(
                        out=a_r, in0=a_r, scalar1=GAMMA, scalar2=None, op0=Alu.mult,
                    )
                    nega = small.tile([128, 1], f32, name="nega")
                    nc.vector.tensor_scalar(
                        out=nega, in0=a_r, scalar1=-1.0, scalar2=None, op0=Alu.mult,
                    )
                    mua = small.tile([128, 1], f32, name="mua")
                    nc.vector.tensor_mul(mua, mvall[:, g, 0:1], a_r)
                    pn = mats.tile([128, N], f32, name="pn", bufs=1)
                    nc.scalar.activation(
                        out=pn, in_=q_sb[g], func=Act.Identity, bias=mua, scale=nega,
                    )
                    nc.vector.max(out=s16_all[:, g, 0:8], in_=pn)
                    pn2 = mats.tile([128, N], f32, name="pn2", bufs=1)
                    nc.vector.match_replace(
                        out=pn2, in_to_replace=s16_all[:, g, 0:8], in_values=pn,
                        imm_value=-1e30,
                    )
                    nc.vector.max(out=s16_all[:, g, 8:16], in_=pn2)

            # ============ phase 2: batched entmax threshold ============
            zs = s16_all[:, :, 1:K]               # [128, NG, KD] sorted off-diag
            rm3 = s16_all[:, :, 1:2]              # [128, NG, 1] row max
            xk = stats.tile([128, NG, KD], f32, name="xk", bufs=1)
            nc.vector.tensor_sub(xk, zs, rm3.to_broadcast([128, NG, KD]))
            nc.vector.tensor_scalar(
                out=xk, in0=xk, scalar1=0.5, scalar2=None, op0=Alu.mult,
            )
            xk2 = stats.tile([128, NG, KD], f32, name="xk2", bufs=1)
            nc.vector.tensor_mul(xk2, xk, xk)
            cs1 = stats.tile([128, NG, KD], f32, name="cs1", bufs=1)
            cs2 = stats.tile([128, NG, KD], f32, name="cs2", bufs=1)
            for g in range(NG):
                nc.vector.tensor_tensor_scan(
                    out=cs1[:, g, :], data0=xk[:, g, :], data1=xk[:, g, :],
                    initial=0.0, op0=Alu.add, op1=Alu.bypass,
                )
                nc.vector.tensor_tensor_scan(
                    out=cs2[:, g, :], data0=xk2[:, g, :], data1=xk2[:, g, :],
                    initial=0.0, op0=Alu.add, op1=Alu.bypass,
                )
            m_t = stats.tile([128, NG, KD], f32, name="m_t", bufs=1)
            nc.vector.tensor_mul(m_t, cs1, irho_sb)
            msq = stats.tile([128, NG, KD], f32, name="msq", bufs=1)
            nc.vector.tensor_mul(msq, cs2, irho_sb)
            mm_ = stats.tile([128, NG, KD], f32, name="mm_", bufs=1)
            nc.vector.tensor_mul(mm_, m_t, m_t)
            ss_ = stats.tile([128, NG, KD], f32, name="ss_", bufs=1)
            nc.vector.tensor_sub(ss_, msq, mm_)
            nc.vector.tensor_mul(ss_, ss_, rho_sb)
            dl = stats.tile([128, NG, KD], f32, name="dl", bufs=1)
            nc.scalar.activation(out=dl, in_=ss_, func=Act.Identity, bias=1.0, scale=-1.0)
            nc.vector.tensor_mul(dl, dl, irho_sb)
            nc.vector.tensor_scalar(
                out=dl, in0=dl, scalar1=0.0, scalar2=None, op0=Alu.max,
            )
            sq_d = stats.tile([128, NG, KD], f32, name="sq_d", bufs=1)
            nc.scalar.activation(out=sq_d, in_=dl, func=Act.Sqrt)
            tau = stats.tile([128, NG, KD], f32, name="tau", bufs=1)
            nc.vector.tensor_sub(tau, m_t, sq_d)
            ind = stats.tile([128, NG, KD], f32, name="ind", bufs=1)
            nc.vector.tensor_tensor(out=ind, in0=tau, in1=xk, op=Alu.is_le)
            dsel = stats.tile([128, NG, KD], f32, name="dsel", bufs=1)
            nc.vector.tensor_sub(
                dsel[:, :, 0:KD - 1], ind[:, :, 0:KD - 1], ind[:, :, 1:KD],
            )
            nc.vector.tensor_copy(out=dsel[:, :, KD - 1:KD], in_=ind[:, :, KD - 1:KD])
            tsel = stats.tile([128, NG, KD], f32, name="tsel", bufs=1)
            nc.vector.tensor_mul(tsel, tau, dsel)
            tau_s = stats.tile([128, NG], f32, name="tau_s", bufs=1)
            nc.vector.tensor_reduce(
                out=tau_s, in_=tsel, axis=mybir.AxisListType.X, op=Alu.add,
            )
            u_t = stats.tile([128, NG], f32, name="u_t", bufs=1)
            nc.vector.tensor_scalar(
                out=u_t, in0=tau_s, scalar1=2.0, scalar2=None, op0=Alu.mult,
            )
            nc.vector.tensor_add(u_t, u_t, rm3.rearrange("p g one -> p (g one)"))
            nc.vector.tensor_mul(u_t, u_t, sig_all)
            nc.vector.tensor_scalar(
                out=u_t, in0=u_t, scalar1=-1.0 / GAMMA, scalar2=None, op0=Alu.mult,
            )
            nc.vector.tensor_add(
                qthr_all, u_t, mvall[:, :, 0].rearrange("p g -> p g"),
            )
            nc.gpsimd.dma_start(
                out=bass.AP(tensor=cc2_in[:].tensor, offset=cc2_in[:].offset,
                            ap=[[1, 128], [128, NG]]),
                in_=qthr_all,
            )
            nc.gpsimd.collective_compute(
                "AllGather", Alu.bypass,
                replica_groups=[list(range(NCORES))],
                ins=[cc2_in[:].opt()], outs=[cc2_out[:].opt()],
            )

            # ============ phase 3: adjacency + logprobs ============
            with tc.tile_pool(name="ph3", bufs=2) as ph3:
                qthr_b = ph3.tile([128, N], f32, name="qthr_b", bufs=1)
                nc.gpsimd.dma_start(
                    out=qthr_b,
                    in_=bass.AP(tensor=cc2_out[:].tensor, offset=cc2_out[:].offset,
                                ap=[[0, 128], [1, N]]),
                )
                for g in range(NG):
                    thr = ph3.tile([128, N], f32, name="thr", bufs=1)
                    nc.vector.tensor_scalar(
                        out=thr, in0=qthr_b, scalar1=qthr_all[:, g:g + 1],
                        scalar2=None, op0=Alu.max,
                    )
                    b_f = ph3.tile([128, N], f32, name="b_f")
                    nc.vector.tensor_tensor(out=b_f, in0=q_sb[g], in1=thr, op=Alu.is_lt)
                    adj_t = ph3.tile([128, N], i32, name="adj_t")
                    lp_f = stats.tile([128, 1], f32, name="lp_f")
                    nc.scalar.activation(
                        out=adj_t, in_=b_f, func=Act.Copy, accum_out=lp_f,
                    )
                    lp_i = stats.tile([128, 1], i32, name="lp_i")
                    nc.vector.tensor_copy(out=lp_i, in_=lp_f)
                    nc.sync.dma_start(out=adj_out[128 * g:128 * (g + 1), :], in_=adj_t)
                    nc.sync.dma_start(out=lp_out[128 * g:128 * (g + 1), :], in_=lp_i)

    nc.finalize()
    return nc


def _get_nc():
    if "nc" not in _BUILT:
        _BUILT["nc"] = _build_nc()
    return _BUILT["nc"]


def _make_bundle(xT, xTm, W_enc, W_emb):
    bun = np.zeros((D, C_TOT), dtype=np.float32)
    bun[:, C_XT:C_XT + N] = xT
    bun[:, C_XTM:C_XTM + RB] = xTm
    bun[:, C_WENC:C_WENC + H] = W_enc
    bun[:, C_WEMB:C_WEMB + H] = W_emb
    bun[0:128, C_EYE:C_EYE + 128] = np.eye(128, dtype=np.float32) * np.float32(-0.5)
    bun[:, C_RHO:C_RHO + NG * KD] = np.tile(
        np.arange(1, KD + 1, dtype=np.float32), NG)[None, :]
    return bun


def kernel(**inputs):
    x = np.ascontiguousarray(np.asarray(inputs["x"], dtype=np.float32))
    W_enc = np.ascontiguousarray(np.asarray(inputs["W_enc"], dtype=np.float32))
    W_emb = np.ascontiguousarray(np.asarray(inputs["W_emb"], dtype=np.float32))
    assert x.shape == (N, D)

    from concourse.bass_utils import run_bass_kernel_spmd

    xT = np.ascontiguousarray(x.T)
    in_maps = []
    for c in range(NCORES):
        in_maps.append({
            "bun": _make_bundle(xT, xT[:, c * RB:(c + 1) * RB], W_enc, W_emb),
        })

    nc = _get_nc()
    res = run_bass_kernel_spmd(nc, in_maps, core_ids=list(range(NCORES)))
    _BUILT["last_results"] = res
    outs = res.results

    x_ = np.concatenate([outs[c]["x_out"] for c in range(NCORES)], axis=0)
    x_aux = np.concatenate([outs[c]["xa_out"] for c in range(NCORES)], axis=0)
    adj = np.concatenate([outs[c]["adj_out"] for c in range(NCORES)], axis=0)
    lp = np.concatenate([outs[c]["lp_out"] for c in range(NCORES)], axis=0).reshape(N)

    diag = np.ascontiguousarray(np.diagonal(adj)).astype(np.int32)
    logprobs = (lp - diag).astype(np.int32)
    np.fill_diagonal(adj, 0)
    return x_, x_aux, adj, logprobs
